# revision 6
# baseline (speedup 1.0000x reference)
"""Trainium2 Bass kernel for DTransformer sparse-attention layer (8 NeuronCores).

Sharding: each core owns a 128-row slice of the sequence axis (both batches,
all heads). Each core computes full K/V projections (replicated), its rows'
attention + distance-decay + maxout + k-select sparse softmax, its out-proj
rows, residual + LayerNorm rows. Host concatenates the 8 row-shards.
"""
import numpy as np

import concourse.bass as bass
import concourse.tile as tile
from concourse import bacc
from concourse import mybir
from concourse.bass_utils import run_bass_kernel_spmd

F32 = mybir.dt.float32
B, S, D, H = 2, 1024, 256, 8
DK = D // H            # 32
NCORES = 8
RP = S // NCORES       # 128 rows per core
K_INDEX = 5
LN_EPS = 1e-5
NEG = -1.0e30
AF = mybir.ActivationFunctionType
ALU = mybir.AluOpType
AX = mybir.AxisListType

_CACHE = {}


def _build_nc():
    nc = bacc.Bacc("TRN2", target_bir_lowering=False, debug=False)

    # ---- DRAM I/O (per-core SPMD tensors) ----
    key_d = nc.dram_tensor("keyx", [B, S, D], F32, kind="ExternalInput")
    val_d = nc.dram_tensor("valx", [B, S, D], F32, kind="ExternalInput")
    qrow_d = nc.dram_tensor("qrow", [B, RP, D], F32, kind="ExternalInput")
    qbo_d = nc.dram_tensor("qbo", [B, RP, D], F32, kind="ExternalInput")
    wqt_s_d = nc.dram_tensor("wqt_s", [D, D], F32, kind="ExternalInput")
    wqt_d = nc.dram_tensor("wqt", [D, D], F32, kind="ExternalInput")
    wvt_d = nc.dram_tensor("wvt", [D, D], F32, kind="ExternalInput")
    wot_d = nc.dram_tensor("wot", [DK, H * D], F32, kind="ExternalInput")
    bqs_d = nc.dram_tensor("bqs", [1, D], F32, kind="ExternalInput")
    bqv_d = nc.dram_tensor("bqv", [1, D], F32, kind="ExternalInput")
    bvv_d = nc.dram_tensor("bvv", [1, D], F32, kind="ExternalInput")
    lnw_d = nc.dram_tensor("lnw2", [RP, D], F32, kind="ExternalInput")
    lnb_d = nc.dram_tensor("lnb2", [RP, D], F32, kind="ExternalInput")
    gneg_d = nc.dram_tensor("gneg", [RP, H], F32, kind="ExternalInput")
    posm_d = nc.dram_tensor("posm", [RP, S], F32, kind="ExternalInput")
    maskb_d = nc.dram_tensor("maskb", [RP, S], F32, kind="ExternalInput")
    throv_d = nc.dram_tensor("throv", [RP, 1], F32, kind="ExternalInput")
    row0m_d = nc.dram_tensor("row0m", [RP, 1], F32, kind="ExternalInput")
    ident_d = nc.dram_tensor("ident", [128, 128], F32, kind="ExternalInput")

    y_d = nc.dram_tensor("y_out", [B, RP, D], F32, kind="ExternalOutput")
    sp_d = nc.dram_tensor("sp_out", [B, H, RP, S], F32, kind="ExternalOutput")

    with tile.TileContext(nc) as tc:
        with (
            tc.tile_pool(name="const", bufs=1) as cpool,
            tc.tile_pool(name="persist", bufs=1) as ppool,
            tc.tile_pool(name="stage", bufs=2) as spool,
            tc.tile_pool(name="small", bufs=3) as smp,
            tc.tile_pool(name="ps_sc", bufs=2, space="PSUM") as ps_sc,
            tc.tile_pool(name="ps_tr", bufs=1, space="PSUM") as ps_tr,
            tc.tile_pool(name="ps_ms", bufs=2, space="PSUM") as ps_ms,
        ):
            # ---- constants (weights as 2 x [128, D] contraction chunks) ----
            def wtiles(tag, dram):
                ts = []
                for kc in range(2):
                    t = cpool.tile([128, D], F32, tag=f"{tag}{kc}")
                    nc.sync.dma_start(t[:], dram[kc * 128:(kc + 1) * 128, :])
                    ts.append(t)
                return ts

            wqt_s = wtiles("wqts", wqt_s_d)
            wqt = wtiles("wqt", wqt_d)
            wvt = wtiles("wvt", wvt_d)
            wot = cpool.tile([DK, H * D], F32, tag="wot")
            nc.sync.dma_start(wot[:], wot_d[:])
            bqs = cpool.tile([1, D], F32, tag="bqs")
            bqv = cpool.tile([1, D], F32, tag="bqv")
            bvv = cpool.tile([1, D], F32, tag="bvv")
            lnw = cpool.tile([RP, D], F32, tag="lnw")
            lnb = cpool.tile([RP, D], F32, tag="lnb")
            gneg = cpool.tile([RP, H], F32, tag="gneg")
            posm = cpool.tile([RP, S], F32, tag="posm")
            maskb = cpool.tile([RP, S], F32, tag="maskb")
            throv = cpool.tile([RP, 1], F32, tag="throv")
            row0m = cpool.tile([RP, 1], F32, tag="row0m")
            ident = cpool.tile([128, 128], F32, tag="ident")
            for t, d in ((bqs, bqs_d), (bqv, bqv_d), (bvv, bvv_d), (lnw, lnw_d),
                         (lnb, lnb_d), (gneg, gneg_d), (posm, posm_d),
                         (maskb, maskb_d), (throv, throv_d), (row0m, row0m_d),
                         (ident, ident_d)):
                nc.sync.dma_start(t[:], d[:])
            ones = cpool.tile([1, S], F32, tag="ones")
            nc.vector.memset(ones[:], 1.0)

            for b in range(B):
                # =========== Stage B: projections for batch b ===========
                qr = spool.tile([RP, D], F32, tag="qr")
                nc.sync.dma_start(qr[:], qrow_d[b])
                qTt = []
                for dh in range(2):
                    tps = ps_tr.tile([128, 128], F32, tag="tr")
                    nc.tensor.transpose(tps[:], qr[:, dh * 128:(dh + 1) * 128], ident[:])
                    qT_in = spool.tile([128, RP], F32, tag=f"qTin{dh}")
                    nc.scalar.activation(qT_in[:], tps[:], AF.Copy)
                    qTt.append(qT_in)
                qT = []
                for g in range(4):
                    mps = ps_ms.tile([64, RP], F32, tag="ms")
                    for kc in range(2):
                        nc.tensor.matmul(mps[:], wqt_s[kc][:, g * 64:(g + 1) * 64],
                                         qTt[kc][:], start=(kc == 0), stop=False)
                    nc.tensor.matmul(mps[:], bqs[0:1, g * 64:(g + 1) * 64],
                                     ones[0:1, 0:RP], start=False, stop=True)
                    qh = ppool.tile([64, RP], F32, tag=f"qT{b}{g}", name=f"qT{b}{g}")
                    nc.scalar.activation(qh[:], mps[:], AF.Copy)
                    qT.append(qh)

                # keyT (transient) and kT[b] = Wq.T.T @ keyT + bq
                keyT = [spool.tile([128, S], F32, tag=f"keyT{kc}", name=f"keyT{kc}") for kc in range(2)]
                for st in range(8):
                    ks = spool.tile([128, D], F32, tag="kstrip")
                    nc.sync.dma_start(ks[:], key_d[b, st * 128:(st + 1) * 128, :])
                    for dh in range(2):
                        tps = ps_tr.tile([128, 128], F32, tag="tr")
                        nc.tensor.transpose(tps[:], ks[:, dh * 128:(dh + 1) * 128], ident[:])
                        nc.scalar.activation(keyT[dh][:, st * 128:(st + 1) * 128], tps[:], AF.Copy)
                kT = []
                for g in range(4):
                    kthis = ppool.tile([64, S], F32, tag=f"kT{b}{g}", name=f"kT{b}{g}")
                    for nch in range(2):
                        mps = ps_ms.tile([64, 512], F32, tag="ms")
                        for kc in range(2):
                            nc.tensor.matmul(mps[:], wqt[kc][:, g * 64:(g + 1) * 64],
                                             keyT[kc][:, nch * 512:(nch + 1) * 512],
                                             start=(kc == 0), stop=False)
                        nc.tensor.matmul(mps[:], bqv[0:1, g * 64:(g + 1) * 64],
                                         ones[0:1, 0:512], start=False, stop=True)
                        nc.scalar.activation(kthis[:, nch * 512:(nch + 1) * 512], mps[:], AF.Copy)
                    kT.append(kthis)

                # valuesT (transient) then v natural [S, 256] strips + bv
                valT = [spool.tile([128, S], F32, tag=f"valT{kc}", name=f"valT{kc}") for kc in range(2)]
                for st in range(8):
                    vs = spool.tile([RP, D], F32, tag="vstrip")
                    nc.sync.dma_start(vs[:], val_d[b, st * 128:(st + 1) * 128, :])
                    for dh in range(2):
                        tps = ps_tr.tile([128, 128], F32, tag="tr")
                        nc.tensor.transpose(tps[:], vs[:, dh * 128:(dh + 1) * 128], ident[:])
                        nc.scalar.activation(valT[dh][:, st * 128:(st + 1) * 128], tps[:], AF.Copy)
                vsb = []
                for st in range(8):
                    mps = ps_ms.tile([128, D], F32, tag="ms")
                    for kc in range(2):
                        nc.tensor.matmul(mps[:], valT[kc][:, st * 128:(st + 1) * 128],
                                         wvt[kc][:], start=(kc == 0), stop=False)
                    nc.tensor.matmul(mps[:], ones[0:1, 0:128], bvv[0:1, :],
                                     start=False, stop=True)
                    vt = ppool.tile([128, D], F32, tag=f"v{b}{st}")
                    nc.scalar.activation(vt[:], mps[:], AF.Copy)
                    vsb.append(vt)

                # =========== Stage C: attention rows for (b, h) ===========
                xacc = ppool.tile([RP, D], F32, tag=f"xacc{b}")
                for h in range(H):
                    g, hb = h // 2, (h % 2) * 32
                    scp = ps_sc.tile([RP, S], F32, tag="sc")
                    for nch in range(2):
                        nc.tensor.matmul(scp[:, nch * 512:(nch + 1) * 512],
                                         qT[g][hb:hb + 32, :],
                                         kT[g][hb:hb + 32, nch * 512:(nch + 1) * 512],
                                         start=True, stop=True)
                    sm = spool.tile([RP, S], F32, tag="sm")
                    nc.vector.tensor_add(sm[:], scp[:], maskb[:])
                    mneg = smp.tile([RP, 1], F32, tag="mneg")
                    nc.vector.tensor_reduce(mneg[:], sm[:], axis=AX.X, op=ALU.max, negate=True)
                    ep = spool.tile([RP, S], F32, tag="ep")
                    nc.scalar.activation(ep[:], sm[:], AF.Exp, bias=mneg[:], scale=1.0)
                    cum = spool.tile([RP, S], F32, tag="cum")
                    nc.vector.tensor_tensor_scan(cum[:], ep[:], ep[:], 0.0, ALU.add, ALU.bypass)
                    invp = smp.tile([RP, 1], F32, tag="invp")
                    nc.vector.reciprocal(invp[:], cum[:, S - 1:S])
                    # u = sum - cum  (>=0, reuse ep buffer), then u *= pos
                    nc.scalar.activation(ep[:], cum[:], AF.Identity, bias=cum[:, S - 1:S], scale=-1.0)
                    nc.vector.tensor_mul(ep[:], ep[:], posm[:])
                    # dist = sqrt(u * invp)  (reuse cum buffer)
                    nc.scalar.activation(cum[:], ep[:], AF.Sqrt, bias=0.0, scale=invp[:])
                    te = spool.tile([RP, S], F32, tag="te")
                    nc.scalar.activation(te[:], cum[:], AF.Exp, bias=0.0, scale=gneg[:, h:h + 1])
                    nc.gpsimd.tensor_scalar_max(te[:], te[:], 1e-5)
                    nc.vector.tensor_mul(sm[:], sm[:], te[:])
                    m2neg = smp.tile([RP, 1], F32, tag="m2neg")
                    nc.vector.tensor_reduce(m2neg[:], sm[:], axis=AX.X, op=ALU.max, negate=True)
                    s2sum = smp.tile([RP, 1], F32, tag="s2sum")
                    nc.scalar.activation(ep[:], sm[:], AF.Exp, bias=m2neg[:], scale=1.0,
                                         accum_out=s2sum[:])  # e2 in ep
                    # c = min(1, 5/sum); c' = c*row0m; cneg = -c
                    rr = smp.tile([RP, 1], F32, tag="rr")
                    nc.vector.reciprocal(rr[:], s2sum[:])
                    dd = smp.tile([RP, 1], F32, tag="dd")
                    nc.vector.tensor_scalar_max(dd[:], rr[:], 0.2)
                    ddi = smp.tile([RP, 1], F32, tag="ddi")
                    nc.vector.reciprocal(ddi[:], dd[:])
                    cc = smp.tile([RP, 1], F32, tag="cc")
                    nc.vector.tensor_mul(cc[:], rr[:], ddi[:])
                    cneg = smp.tile([RP, 1], F32, tag="cneg")
                    nc.vector.tensor_scalar_mul(cneg[:], cc[:], -1.0)
                    cp = smp.tile([RP, 1], F32, tag="cp")
                    nc.vector.tensor_mul(cp[:], cc[:], row0m[:])
                    attf = spool.tile([RP, S], F32, tag="attf")
                    nc.scalar.activation(attf[:], ep[:], AF.Copy, bias=0.0, scale=cp[:])
                    # --- sparse branch: top-5 threshold + softmax ---
                    t8 = smp.tile([RP, 8], F32, tag="t8")
                    nc.vector.max(t8[:], attf[:])
                    thr = smp.tile([RP, 1], F32, tag="thr")
                    nc.vector.tensor_scalar(thr[:], t8[:, 4:5], throv[:], None, ALU.min)
                    ge = spool.tile([RP, S], F32, tag="ge")
                    nc.gpsimd.tensor_scalar(ge[:], attf[:], thr[:], None, ALU.is_ge)
                    qe = spool.tile([RP, S], F32, tag="qe")
                    nc.scalar.activation(qe[:], attf[:], AF.Exp, bias=cneg[:], scale=1.0)
                    nc.vector.tensor_mul(qe[:], qe[:], ge[:])
                    ssum = smp.tile([RP, 1], F32, tag="ssum")
                    nc.vector.tensor_reduce(ssum[:], qe[:], axis=AX.X, op=ALU.add)
                    sinv = smp.tile([RP, 1], F32, tag="sinv")
                    nc.vector.reciprocal(sinv[:], ssum[:])
                    # sparse out (reuse ge buffer)
                    nc.scalar.activation(ge[:], qe[:], AF.Copy, bias=0.0, scale=sinv[:])
                    nc.sync.dma_start(sp_d[b, h], ge[:])
                    # --- att @ v then out-proj rank-update ---
                    trp = ps_tr.tile([RP, S], F32, tag="tr")
                    for blk in range(8):
                        nc.tensor.transpose(trp[:, blk * 128:(blk + 1) * 128],
                                            attf[:, blk * 128:(blk + 1) * 128], ident[:])
                    attT = spool.tile([RP, S], F32, tag="attT")
                    nc.scalar.activation(attT[:], trp[:], AF.Copy)
                    otp = ps_ms.tile([32, 128], F32, tag="ms")
                    for blk in range(8):
                        nc.tensor.matmul(otp[:], vsb[blk][:, h * 32:h * 32 + 32],
                                         attT[:, blk * 128:(blk + 1) * 128],
                                         start=(blk == 0), stop=(blk == 7))
                    ots = spool.tile([32, 128], F32, tag="ots")
                    nc.scalar.activation(ots[:], otp[:], AF.Copy)
                    opp = ps_ms.tile([RP, D], F32, tag="ms")
                    nc.tensor.matmul(opp[:], ots[:], wot[:, h * D:(h + 1) * D], start=True, stop=True)
                    if h == 0:
                        qbo_sb = spool.tile([RP, D], F32, tag="qbo")
                        nc.sync.dma_start(qbo_sb[:], qbo_d[b])
                        nc.vector.tensor_add(xacc[:], qbo_sb[:], opp[:])
                    else:
                        nc.vector.tensor_add(xacc[:], xacc[:], opp[:])

                # =========== Stage D: residual LayerNorm rows ===========
                bst = smp.tile([RP, 6], F32, tag="bst")
                nc.vector.bn_stats(bst[:], xacc[:])
                mv = smp.tile([RP, 2], F32, tag="mv")
                nc.vector.bn_aggr(mv[:], bst[:])
                veps = smp.tile([RP, 1], F32, tag="veps")
                nc.vector.tensor_scalar_add(veps[:], mv[:, 1:2], LN_EPS)
                sd = smp.tile([RP, 1], F32, tag="sd")
                nc.scalar.activation(sd[:], veps[:], AF.Sqrt)
                isd = smp.tile([RP, 1], F32, tag="isd")
                nc.vector.reciprocal(isd[:], sd[:])
                nmu = smp.tile([RP, 1], F32, tag="nmu")
                nc.vector.tensor_mul(nmu[:], mv[:, 0:1], isd[:])
                nc.vector.tensor_scalar_mul(nmu[:], nmu[:], -1.0)
                z = spool.tile([RP, D], F32, tag="z")
                nc.scalar.activation(z[:], xacc[:], AF.Identity, bias=nmu[:], scale=isd[:])
                nc.vector.tensor_mul(z[:], z[:], lnw[:])
                nc.vector.tensor_add(z[:], z[:], lnb[:])
                nc.sync.dma_start(y_d[b], z[:])

    nc.compile()
    return nc


def _prep_inputs(inputs):
    """Host-side prep: weights transposes + per-core constant strips."""
    gi = {k: np.asarray(v, dtype=np.int32 if k == "lens" else np.float32)
          for k, v in inputs.items()}
    query, Wq, bq = gi["query"], gi["Wq"], gi["bq"]
    Wv, bv, Wo, bo = gi["Wv"], gi["bv"], gi["Wo"], gi["bo"]
    gammas, ln_w, ln_b = gi["gammas"], gi["ln_w"], gi["ln_b"]
    sc = np.float32(1.0 / np.sqrt(np.float32(DK)))

    shared = {
        "keyx": gi["key"],
        "valx": gi["values"],
        "wqt_s": np.ascontiguousarray((Wq * sc).T),
        "wqt": np.ascontiguousarray(Wq.T),
        "wvt": np.ascontiguousarray(Wv.T),
        "wot": np.ascontiguousarray(
            Wo.T.reshape(H, DK, D).transpose(1, 0, 2).reshape(DK, H * D)),
        "bqs": (bq * sc).reshape(1, D),
        "bqv": bq.reshape(1, D),
        "bvv": bv.reshape(1, D),
        "lnw2": np.ascontiguousarray(np.tile(ln_w.reshape(1, D), (RP, 1))),
        "lnb2": np.ascontiguousarray(np.tile(ln_b.reshape(1, D), (RP, 1))),
        "gneg": np.ascontiguousarray(
            np.tile((-np.abs(gammas[:, 0, 0])).reshape(1, H), (RP, 1))),
        "ident": np.eye(128, dtype=np.float32),
    }
    in_maps = []
    j = np.arange(S, dtype=np.float32)
    for c in range(NCORES):
        i = np.arange(c * RP, (c + 1) * RP, dtype=np.float32)
        posm = np.abs(i[:, None] - j[None, :]).astype(np.float32)
        maskb = np.where(j[None, :] < i[:, None], 0.0, NEG).astype(np.float32)
        ig = np.arange(c * RP, (c + 1) * RP)
        throv = np.where(ig < K_INDEX, -1e30, 1e30).astype(np.float32).reshape(RP, 1)
        row0m = (ig != 0).astype(np.float32).reshape(RP, 1)
        qrow = np.ascontiguousarray(query[:, c * RP:(c + 1) * RP, :])
        in_maps.append({
            **shared,
            "qrow": qrow,
            "qbo": qrow + bo.reshape(1, 1, D),
            "posm": posm, "maskb": maskb, "throv": throv, "row0m": row0m,
        })
    return in_maps


def kernel(**inputs):
    if "nc" not in _CACHE:
        _CACHE["nc"] = _build_nc()
    nc = _CACHE["nc"]
    in_maps = _prep_inputs(inputs)
    res = run_bass_kernel_spmd(nc, in_maps, core_ids=list(range(NCORES)),
                               trace=bool(_CACHE.get("trace", False)))
    _CACHE["last_exec_ns"] = res.exec_time_ns
    y = np.concatenate([res.results[c]["y_out"] for c in range(NCORES)], axis=1)
    sp = np.concatenate([res.results[c]["sp_out"] for c in range(NCORES)], axis=2)
    return y, sp
